# revision 30
# baseline (speedup 1.0000x reference)
"""Trainium2 Bass kernel for the batched attention module:

    proj   = input @ W.T + b            # [B, TQ, 2H]
    scores = proj @ enc                 # [B, TQ, S]   (enc: [B, 2H, S], S == 2H)
    attn   = softmax(scores, axis=-1)
    out    = attn @ enc                 # [B, TQ, S]

Sharding: data-parallel over batch, one batch per NeuronCore (8 cores).
All matmuls run as float32r (fp32 stored, fp22 multiplied, fp32 accumulated)
which streams at 1 cycle/row on the PE -- 4x the fp32 rate.

Dataflow per core (batch):
  P1:  projT[d,q] = sum_h WT[h,d] * inputT[h,q]  (+bias), per q-group of 256
  P2:  scores[q,s] (q on partitions) accumulated over 16 d-tiles in PSUM
       softmax stats on the free dim: DVE row-max (negated) -> ACT Exp with
       per-partition bias and accumulated row-sum -> DVE reciprocal
  T:   PE-transpose E=[q,s] 128x128 blocks -> ET[s,q]
  P3:  out[q,v] = sum_s ET[s,q].T @ enc[s,v], scaled by 1/rowsum on eviction
"""

import sys

import numpy as np

for _p in ("/opt/trn_rl_repo",):
    if _p not in sys.path:
        sys.path.insert(0, _p)

from concourse import bacc, bass, mybir, tile  # noqa: E402
from concourse.bass_utils import run_bass_kernel_spmd  # noqa: E402
from concourse.masks import make_identity  # noqa: E402

F32 = mybir.dt.float32
F32R = mybir.dt.float32r
AF = mybir.ActivationFunctionType
AX = mybir.AxisListType
ALU = mybir.AluOpType

B = 8
TQ = 1024
H = 1024
D = 2 * H  # 2048, proj feature dim == contraction dim of scores
S = 2 * H  # 2048
P = 128

NHT = H // P  # 8  h-tiles
NDT = D // P  # 16 d-tiles
NST = S // P  # 16 s-tiles
NQT = TQ // P  # 8 q-tiles
QG = 512  # q-group width for the proj phase (moving dim >= 256 for f32r rate)
NG = TQ // QG  # 4 groups
QTPG = QG // P  # 2 q-tiles per group
NCH = 512  # moving-dim chunk for scores/out matmuls (one PSUM bank of fp32)
NSC = S // NCH  # 4


def r32(ap):
    return ap.bitcast(F32R)


def build_program(loop_n: int = 1) -> bass.Bass:
    nc = bacc.Bacc(
        "TRN2",
        target_bir_lowering=False,
        debug=False,
        # default 16KB/partition of SWDGE descriptor scratch; we only use
        # HWDGE queues (sync/scalar), so reclaim most of it for tiles
        dynamic_dma_scratch_size=2048,
    )
    inpT = nc.declare_dram_parameter("inpT", [NHT, P, TQ], F32, isOutput=False)
    wt = nc.declare_dram_parameter("wt", [NDT, NHT, P, P], F32, isOutput=False)
    enc = nc.declare_dram_parameter("enc", [S, S], F32, isOutput=False)
    bvec = nc.declare_dram_parameter("bvec", [P, NDT], F32, isOutput=False)
    out = nc.declare_dram_parameter("out", [TQ, S], F32, isOutput=True)

    with tile.TileContext(nc) as tc:
        with (
            tc.tile_pool(name="const", bufs=1) as constp,
            tc.tile_pool(name="inp", bufs=1) as inpp,
            tc.tile_pool(name="wtp", bufs=4) as wtp,
            tc.tile_pool(name="projp", bufs=1) as projp,
            tc.tile_pool(name="ep", bufs=1) as ep,
            tc.tile_pool(name="etp", bufs=2) as etp,
            tc.tile_pool(name="outp", bufs=1) as outp,
            tc.tile_pool(name="statp", bufs=2) as statp,
            tc.tile_pool(name="ps_sc", bufs=1, space="PSUM") as ps_sc,
            tc.tile_pool(name="ps_small", bufs=2, space="PSUM") as ps_small,
            tc.tile_pool(name="ps_out", bufs=2, space="PSUM") as ps_out,
        ):
            ident = constp.tile([P, P], F32)
            make_identity(nc, ident[:])
            bias_sb = constp.tile([P, NDT], F32)
            nc.sync.dma_start(out=bias_sb[:], in_=bvec[:])

            import contextlib

            loop_ctx = (
                tc.For_i(0, loop_n, 1, hint_engines=(mybir.EngineType.PE,))
                if loop_n > 1
                else contextlib.nullcontext()
            )
            loop_ctx.__enter__()

            def emit_p1_dt(g, dt_, projT, inp_g, dma):
                """One d-tile of the proj phase: wt DMA + 8 matmuls + evict."""
                wt_sl = wtp.tile([P, NHT, P], F32R, tag="wt")
                dma.dma_start(out=wt_sl[:], in_=r32(wt[dt_].transpose([1, 0, 2])))
                pp = ps_small.tile([P, QG], F32, tag="small")
                for ht in range(NHT):
                    nc.tensor.matmul(
                        pp[:],
                        wt_sl[:, ht, :],
                        inp_g[:, ht, :],
                        start=(ht == 0),
                        stop=(ht == NHT - 1),
                    )
                nc.scalar.add(projT[:, dt_, :], pp[:], bias_sb[:, dt_ : dt_ + 1])

            def emit_inp_load(g, dma):
                inp_g = inpp.tile([P, NHT, QG], F32R, tag="inp")
                dma.dma_start(
                    out=inp_g[:],
                    in_=r32(inpT[:, :, g * QG : (g + 1) * QG].transpose([1, 0, 2])),
                )
                return inp_g

            def emit_p1(g):
                """projT for one q-group; later groups stream W on the ACT
                queue, gated behind the previous group's matmuls by wt-slot
                reuse, so they never compete with the front loads."""
                dma = nc.scalar
                inp_g = emit_inp_load(g, dma)
                projT = projp.tile([P, NDT, QG], F32R, tag="projT")
                for dt_ in range(NDT):
                    emit_p1_dt(g, dt_, projT, inp_g, dma)
                return projT

            def emit_scores(projT, qloc):
                # dt-outer so each enc tile unlocks 4 matmuls as it arrives
                # (4 interleaved PSUM accumulation groups, one per bank).
                sc = ps_sc.tile([P, S], F32, tag="sc")
                qs = slice(qloc * P, (qloc + 1) * P)
                for dt_ in range(NDT):
                    for c in range(NSC):
                        cs = slice(c * NCH, (c + 1) * NCH)
                        nc.tensor.matmul(
                            sc[:, cs],
                            projT[:, dt_, qs],
                            enc_sb[dt_][:, cs],
                            start=(dt_ == 0),
                            stop=(dt_ == NDT - 1),
                        )
                return sc

            def emit_softmax(sc):
                st = statp.tile([P, 4], F32, tag="stat")
                nc.vector.tensor_reduce(
                    st[:, 0:1], sc[:], axis=AX.X, op=ALU.max, negate=True
                )
                E = ep.tile([P, S], F32, tag="E")
                nc.scalar.activation(
                    E[:],
                    sc[:],
                    AF.Exp,
                    bias=st[:, 0:1],
                    scale=1.0,
                    accum_out=st[:, 1:2],
                )
                nc.vector.reciprocal(st[:, 2:3], st[:, 1:2])
                return E, st

            def emit_transp(E):
                ET = etp.tile([P, NST, P], F32R, tag="ET")
                for s_ in range(NST):
                    tp = ps_small.tile([P, P], F32, tag="small")
                    nc.tensor.transpose(tp[:], E[:, s_ * P : (s_ + 1) * P], ident[:])
                    nc.vector.tensor_copy(ET[:, s_, :], tp[:])
                return ET

            def emit_out(ET, st, qt):
                for c in range(NSC):
                    cs = slice(c * NCH, (c + 1) * NCH)
                    po = ps_out.tile([P, NCH], F32, tag="po")
                    for s_ in range(NST):
                        nc.tensor.matmul(
                            po[:],
                            ET[:, s_, :],
                            enc_sb[s_][:, cs],
                            start=(s_ == 0),
                            stop=(s_ == NST - 1),
                        )
                    ob = outp.tile([P, NCH], F32, tag="ob")
                    nc.scalar.mul(ob[:], po[:], st[:, 2:3])
                    nc.scalar.dma_start(
                        out=out[qt * P : (qt + 1) * P, cs], in_=ob[:]
                    )

            # Software-pipelined emission: PE order per steady-state q-tile is
            # transp(i), [P1(g+1) at group boundary], scores(i+1), out(i) -- the
            # softmax of i+1 runs on DVE/ACT while PE is busy with out(i).
            # Front: interleave P1(0), enc loads, and scores(0) at d-tile
            # granularity. The front is DMA-bound (inp 2MB + W 8MB + enc 16MB
            # must land); weaving wt[dt], enc[dt] onto one queue in need-order
            # lets PE chew P1 and scores(0) as data arrives.
            _mark(nc, "front")
            projs = {}
            enc_sb = []
            inp_g0 = emit_inp_load(0, nc.sync)
            projT0 = projp.tile([P, NDT, QG], F32R, tag="projT")
            projs[0] = projT0
            cur_sc = ps_sc.tile([P, S], F32, tag="sc")
            for dt_ in range(NDT):
                emit_p1_dt(0, dt_, projs[0], inp_g0, nc.sync)
                e = constp.tile([P, S], F32R, tag=f"enc{dt_}")
                nc.sync.dma_start(
                    out=e[:], in_=r32(enc[dt_ * P : (dt_ + 1) * P, :])
                )
                enc_sb.append(e)
                for c in range(NSC):
                    cs = slice(c * NCH, (c + 1) * NCH)
                    nc.tensor.matmul(
                        cur_sc[:, cs],
                        projs[0][:, dt_, 0:P],
                        e[:, cs],
                        start=(dt_ == 0),
                        stop=(dt_ == NDT - 1),
                    )
            for qt in range(NQT):
                _mark(nc, f"softmax({qt})")
                E, st = emit_softmax(cur_sc)
                _mark(nc, f"transp({qt})")
                ET = emit_transp(E)
                nxt = qt + 1
                if nxt < NQT:
                    g, qloc = divmod(nxt, QTPG)
                    if qloc == 0:
                        _mark(nc, f"P1({g})")
                        projs[g] = emit_p1(g)
                    _mark(nc, f"scores({nxt})")
                    cur_sc = emit_scores(projs[g], qloc)
                _mark(nc, f"out({qt})")
                emit_out(ET, st, qt)
            _mark(nc, "end")
            loop_ctx.__exit__(None, None, None)

    nc.compile()
    return nc


PHASES = []  # (instruction id, label) marks populated during build, for tsim


def _mark(nc, label):
    nm = nc.get_next_instruction_name()  # burns one name; fine
    PHASES.append((int(nm.split("-")[1]), label))


_NC_CACHE = {}


def _get_program(loop_n: int = 1) -> bass.Bass:
    if loop_n not in _NC_CACHE:
        PHASES.clear()
        _NC_CACHE[loop_n] = build_program(loop_n)
    return _NC_CACHE[loop_n]


def _prep_in_maps(input, encoder_output, W, b):
    input = np.ascontiguousarray(input, dtype=np.float32)
    encoder_output = np.ascontiguousarray(encoder_output, dtype=np.float32)
    W = np.ascontiguousarray(W, dtype=np.float32)
    b = np.ascontiguousarray(b, dtype=np.float32)

    # inputT per batch: [H, TQ] tiled as [NHT, P, TQ]
    inpT = np.ascontiguousarray(input.transpose(0, 2, 1)).reshape(B, NHT, P, TQ)
    # W[d, h] -> wt[dt, ht, hp, dj] = W[dt*P+dj, ht*P+hp]
    wt = np.ascontiguousarray(W.reshape(NDT, P, NHT, P).transpose(0, 2, 3, 1))
    bvec = np.ascontiguousarray(b.reshape(NDT, P).T)  # [P, NDT]

    return [
        {"inpT": inpT[i], "wt": wt, "enc": encoder_output[i], "bvec": bvec}
        for i in range(B)
    ]


def run(input, encoder_output, W, b, trace=False, loop_n=1):
    """Returns (out [B, TQ, S] float32, BassKernelResults)."""
    nc = _get_program(loop_n)
    in_maps = _prep_in_maps(input, encoder_output, W, b)
    res = run_bass_kernel_spmd(nc, in_maps, list(range(B)), trace=trace)
    out = np.stack([np.asarray(res.results[i]["out"]) for i in range(B)])
    return out, res


def kernel(input, encoder_output, W, b):
    out, _ = run(input, encoder_output, W, b, trace=False)
    return out
